# revision 41
# baseline (speedup 1.0000x reference)
"""Trainium2 Bass kernel for nn_AdditiveScorer (Bahdanau additive attention scores).

reference:
    q = qs @ Wq                      # [B, LQ, H]
    k = ks @ Wk                      # [B, LK, H]
    scores[b,q,k] = sum_h wv[h] * tanh(q[b,q,h] + k[b,k,h])   # [B, LQ, LK]

Shapes (hardcoded): B=4, LQ=LK=D=512, H=128.

Sharding: B*LQ = 2048 query rows split across 8 cores -> 256 rows/core.
Core c handles batch b = c//2, query rows [256*(c%2), 256*(c%2+1)).
Each core only needs its batch's ks (replicated host-side to the core pair).
No collectives: embarrassingly parallel; host gathers per-core score slices.
qs/ks are transposed host-side (pure layout prep) so no PE transposes are
needed on device.

Per-core pipeline (engines run concurrently, ACT tanh is the roofline:
~113us busy of ~137us total):
  - PE: project to qh^T [H=128p, 256], kh^T [H=128p, 512] (bf16, f32 accum)
  - DVE: per query q, feats[:, j, :] = kh^T + qh^T[:, q]
    (tensor_scalar add with per-partition scalar operand, bf16 2x mode)
  - ACT: tanh over [128, G*512] tiles (1 elem/lane/cycle, groups ramp
    4,4,8,16...16,8,4,4 to hide prologue latency and shorten the tail)
  - PE: scores row q = wv^T @ tanh_feats via masked stationary wv (x) e_lane;
    a segment of queries accumulates into one PSUM [seg, 512] tile (rows
    other than `lane` accumulate exact zeros)
  - DVE: PSUM->SBUF copy per segment, DMA out
"""

import ml_dtypes
import numpy as np

import concourse.tile as tile
from concourse import bacc, mybir
from concourse.bass_utils import run_bass_kernel_spmd

FP32 = mybir.dt.float32
BF16 = mybir.dt.bfloat16

B, LQ, LK, D, H = 4, 512, 512, 512, 128
NCORES = 8
QPC = B * LQ // NCORES      # 256 query rows per core
DCH = D // 128              # contraction chunks (4)
QG = 32                     # queries per PSUM accumulation group

# ACT group sizes: small leading groups start the tanh pipeline early (less
# prologue latency), wide middles amortize the per-instruction overhead, small
# tail groups shorten the post-last-tanh PE drain.
GROUPS = [4, 4, 8] + [16] * 14 + [8, 4, 4]
assert sum(GROUPS) == QPC

# PSUM accumulation segments (query ranges). The last two are 16-wide so the
# final output flush starts earlier.
SEGS = [(s, s + 32) for s in range(0, 224, 32)] + [(224, 240), (240, 256)]
_SEG_OF = {}
for _s0, _s1 in SEGS:
    for _q in range(_s0, _s1):
        _SEG_OF[_q] = (_s0, _s1)


def build_nc():
    nc = bacc.Bacc("TRN2", target_bir_lowering=False, debug=False, num_devices=NCORES)

    # inputs arrive already in device layout: [partition, flat free dim]
    qsT_d = nc.dram_tensor("qsT", [128, DCH * QPC], BF16, kind="ExternalInput").ap()
    ksT_d = nc.dram_tensor("ksT", [128, DCH * LK], BF16, kind="ExternalInput").ap()
    wqk_d = nc.dram_tensor("Wqk", [128, 2 * DCH * H], BF16, kind="ExternalInput").ap()
    wv_d = nc.dram_tensor("wv", [H, 1], FP32, kind="ExternalInput").ap()
    out_d = nc.dram_tensor("out", [QPC, LK], FP32, kind="ExternalOutput").ap()

    with tile.TileContext(nc) as tc:
        with (
            tc.tile_pool(name="const", bufs=1) as constp,
            tc.tile_pool(name="proj", bufs=1) as projp,
            tc.tile_pool(name="feats", bufs=4) as featsp,
            tc.tile_pool(name="tanhp", bufs=3) as tanhp,
            tc.tile_pool(name="outs", bufs=2) as outp,
            tc.tile_pool(name="ps_proj", bufs=1, space="PSUM") as ps_proj,
            tc.tile_pool(name="ps_out", bufs=2, space="PSUM") as ps_out,
            tc.tile_pool(name="ps_warm", bufs=1, space="PSUM") as ps_warm,
        ):
            # ---------------- loads (triggers spread across engines) --------
            # ksT is the critical path (kh feeds every bias-add); issue first.
            ksT = projp.tile([128, DCH, LK], BF16)
            nc.sync.dma_start(ksT[:].rearrange("p c k -> p (c k)"), ksT_d)
            qsT = projp.tile([128, DCH, QPC], BF16)
            nc.scalar.dma_start(qsT[:].rearrange("p c q -> p (c q)"), qsT_d)
            wqk_sb = constp.tile([128, 2, DCH, H], BF16)
            nc.gpsimd.dma_start(
                wqk_sb[:].rearrange("p w c h -> p (w c h)"), wqk_d
            )
            wq_sb = wqk_sb[:, 0]
            wk_sb = wqk_sb[:, 1]
            wv_sb = constp.tile([128, 1], FP32)
            nc.gpsimd.dma_start(wv_sb[:], wv_d)

            # masks: msk[:, j, :] = wv (x) e_j -- stationary [128h, 32m] whose
            # only nonzero column is j.
            mskd = constp.tile([128, QG, QG], FP32)
            nc.gpsimd.memset(mskd[:], 0.0)
            nc.gpsimd.affine_select(
                out=mskd[:], in_=mskd[:],
                compare_op=mybir.AluOpType.not_equal, fill=1.0,
                base=0, pattern=[[1, QG], [-1, QG]], channel_multiplier=0,
            )
            msk = constp.tile([128, QG, QG], BF16)
            nc.vector.tensor_scalar_mul(msk[:], mskd[:], wv_sb[:, 0:1])

            # PE warm-up: the tensor engine p-state ramps only after ~3us of
            # continuous work; dummy matmuls on a scratch bank while the input
            # DMAs stream get the projections to full clock.
            dum = constp.tile([128, LK], BF16)
            nc.vector.memset(dum[:], 0.0)
            ps_w = ps_warm.tile([128, LK], FP32)
            for _ in range(9):
                nc.tensor.matmul(dum_out := ps_w[:], dum[:, 0:128], dum[:],
                                 start=True, stop=True)

            # ---------------- project: qh^T = Wq^T qs^T, kh^T = Wk^T ks^T ----
            # bf16 inputs (converted host-side) run the PE at full rate.
            # kh first (every bias-add reads all of kh); the qh psum->sbuf
            # copy is split so the first groups' columns are ready early.
            ps_kh = ps_proj.tile([128, LK], FP32)
            kh_bf = projp.tile([128, LK], BF16)
            for j in range(DCH):
                nc.tensor.matmul(
                    ps_kh[:], wk_sb[:, j, :], ksT[:, j, :],
                    start=(j == 0), stop=(j == DCH - 1),
                )
            nc.vector.tensor_copy(kh_bf[:], ps_kh[:])

            ps_qh = ps_proj.tile([128, QPC], FP32)
            for j in range(DCH):
                nc.tensor.matmul(
                    ps_qh[:], wq_sb[:, j, :], qsT[:, j, :],
                    start=(j == 0), stop=(j == DCH - 1),
                )
            qh_f = projp.tile([128, QPC], FP32)
            nc.vector.tensor_copy(qh_f[:, 0:16], ps_qh[:, 0:16])
            nc.vector.tensor_copy(qh_f[:, 16:], ps_qh[:, 16:])

            # ---------------- main loop: bias-add, tanh, reduce ----------------
            psq = None
            q0 = 0
            for gsize in GROUPS:
                feats = featsp.tile([128, gsize, LK], BF16, name="feats")
                for j in range(gsize):
                    q = q0 + j
                    nc.vector.tensor_scalar_add(
                        feats[:, j, :], kh_bf[:], qh_f[:, q:q + 1]
                    )
                th = tanhp.tile([128, gsize, LK], BF16, name="th")
                nc.scalar.activation(
                    th[:], feats[:], mybir.ActivationFunctionType.Tanh
                )
                for j in range(gsize):
                    q = q0 + j
                    s0, s1 = _SEG_OF[q]
                    m = s1 - s0
                    lane = q - s0
                    if lane == 0:
                        psq = ps_out.tile([m, LK], FP32, name="psq")
                    nc.tensor.matmul(
                        psq[:], msk[:, lane, 0:m], th[:, j, :],
                        start=(lane == 0), stop=(lane == m - 1),
                    )
                    if lane == m - 1:
                        ob = outp.tile([m, LK], FP32, name="ob")
                        nc.vector.tensor_copy(ob[:], psq[:])
                        nc.sync.dma_start(out_d[s0:s1, :], ob[:])
                q0 += gsize

    nc.compile()
    return nc


_NC_CACHE = None


def _get_nc():
    global _NC_CACHE
    if _NC_CACHE is None:
        _NC_CACHE = build_nc()
    return _NC_CACHE


BF16NP = ml_dtypes.bfloat16


def _dev_layout(mT, rows):
    # [D, rows] -> [128, DCH*rows] with partition = d % 128, chunk = d // 128
    return np.ascontiguousarray(
        mT.reshape(DCH, 128, rows).transpose(1, 0, 2).reshape(128, DCH * rows)
    )


def make_in_maps(qs, ks, Wq, Wk, wv):
    qs = np.asarray(qs, dtype=np.float32)
    ks = np.asarray(ks, dtype=np.float32)
    Wqk = np.stack([np.asarray(Wq, dtype=np.float32),
                    np.asarray(Wk, dtype=np.float32)], axis=0).astype(BF16NP)
    # [2, D, H] -> [128, 2*DCH*H]
    Wqk = np.ascontiguousarray(
        Wqk.reshape(2, DCH, 128, H).transpose(2, 0, 1, 3).reshape(128, -1)
    )
    wv = np.ascontiguousarray(np.asarray(wv, dtype=np.float32))
    ksT_b = [_dev_layout(ks[b].T.astype(BF16NP), LK) for b in range(B)]
    in_maps = []
    for c in range(NCORES):
        b, half = divmod(c, 2)
        in_maps.append({
            "qsT": _dev_layout(
                qs[b, half * QPC:(half + 1) * QPC, :].T.astype(BF16NP), QPC
            ),
            "ksT": ksT_b[b],
            "Wqk": Wqk,
            "wv": wv,
        })
    return in_maps


def assemble(results):
    out = np.empty((B, LQ, LK), dtype=np.float32)
    for c in range(NCORES):
        b, half = divmod(c, 2)
        out[b, half * QPC:(half + 1) * QPC, :] = results[c]["out"]
    return out


def run(qs, ks, Wq, Wk, wv, trace=False, tmpdir=None):
    nc = _get_nc()
    in_maps = make_in_maps(qs, ks, Wq, Wk, wv)
    res = run_bass_kernel_spmd(
        nc, in_maps, core_ids=list(range(NCORES)), trace=trace, tmpdir=tmpdir
    )
    return assemble(res.results), res


def kernel(qs, ks, Wq, Wk, wv):
    out, _ = run(qs, ks, Wq, Wk, wv)
    return out


# revision 42
# speedup vs baseline: 1.0043x; 1.0043x over previous
"""Trainium2 Bass kernel for nn_AdditiveScorer (Bahdanau additive attention scores).

reference:
    q = qs @ Wq                      # [B, LQ, H]
    k = ks @ Wk                      # [B, LK, H]
    scores[b,q,k] = sum_h wv[h] * tanh(q[b,q,h] + k[b,k,h])   # [B, LQ, LK]

Shapes (hardcoded): B=4, LQ=LK=D=512, H=128.

Sharding: B*LQ = 2048 query rows split across 8 cores -> 256 rows/core.
Core c handles batch b = c//2, query rows [256*(c%2), 256*(c%2+1)).
Each core only needs its batch's ks (replicated host-side to the core pair).
No collectives: embarrassingly parallel; host gathers per-core score slices.
qs/ks are transposed host-side (pure layout prep) so no PE transposes are
needed on device.

Per-core pipeline (engines run concurrently, ACT tanh is the roofline:
~113us busy of ~137us total):
  - PE: project to qh^T [H=128p, 256], kh^T [H=128p, 512] (bf16, f32 accum)
  - DVE: per query q, feats[:, j, :] = kh^T + qh^T[:, q]
    (tensor_scalar add with per-partition scalar operand, bf16 2x mode)
  - ACT: tanh over [128, G*512] tiles (1 elem/lane/cycle, groups ramp
    4,4,8,16...16,8,4,4 to hide prologue latency and shorten the tail)
  - PE: scores row q = wv^T @ tanh_feats via masked stationary wv (x) e_lane;
    a segment of queries accumulates into one PSUM [seg, 512] tile (rows
    other than `lane` accumulate exact zeros)
  - DVE: PSUM->SBUF copy per segment, DMA out
"""

import ml_dtypes
import numpy as np

import concourse.tile as tile
from concourse import bacc, mybir
from concourse.bass_utils import run_bass_kernel_spmd

FP32 = mybir.dt.float32
BF16 = mybir.dt.bfloat16

B, LQ, LK, D, H = 4, 512, 512, 512, 128
NCORES = 8
QPC = B * LQ // NCORES      # 256 query rows per core
DCH = D // 128              # contraction chunks (4)
QG = 32                     # queries per PSUM accumulation group

# ACT group sizes: small leading groups start the tanh pipeline early (less
# prologue latency), wide middles amortize the per-instruction overhead, small
# tail groups shorten the post-last-tanh PE drain.
GROUPS = [4, 4, 8] + [16] * 14 + [8, 4, 4]
assert sum(GROUPS) == QPC

# PSUM accumulation segments (query ranges). The last two are 16-wide so the
# final output flush starts earlier.
SEGS = [(s, s + 32) for s in range(0, 224, 32)] + [(224, 240), (240, 256)]
_SEG_OF = {}
for _s0, _s1 in SEGS:
    for _q in range(_s0, _s1):
        _SEG_OF[_q] = (_s0, _s1)


def build_nc():
    nc = bacc.Bacc("TRN2", target_bir_lowering=False, debug=False, num_devices=NCORES)

    # inputs arrive already in device layout: [partition, flat free dim]
    qsT_d = nc.dram_tensor("qsT", [128, DCH * QPC], BF16, kind="ExternalInput").ap()
    ksT_d = nc.dram_tensor("ksT", [128, DCH * LK], BF16, kind="ExternalInput").ap()
    wqk_d = nc.dram_tensor("Wqk", [128, 2 * DCH * H], BF16, kind="ExternalInput").ap()
    wv_d = nc.dram_tensor("wv", [H, 1], FP32, kind="ExternalInput").ap()
    out_d = nc.dram_tensor("out", [QPC, LK], FP32, kind="ExternalOutput").ap()

    with tile.TileContext(nc) as tc:
        with (
            tc.tile_pool(name="const", bufs=1) as constp,
            tc.tile_pool(name="proj", bufs=1) as projp,
            tc.tile_pool(name="feats", bufs=4) as featsp,
            tc.tile_pool(name="tanhp", bufs=3) as tanhp,
            tc.tile_pool(name="outs", bufs=2) as outp,
            tc.tile_pool(name="ps_proj", bufs=1, space="PSUM") as ps_proj,
            tc.tile_pool(name="ps_out", bufs=2, space="PSUM") as ps_out,
            tc.tile_pool(name="ps_warm", bufs=1, space="PSUM") as ps_warm,
        ):
            # ---------------- loads (triggers spread across engines) --------
            # ksT is the critical path (kh feeds every bias-add); issue first.
            ksT = projp.tile([128, DCH, LK], BF16)
            nc.sync.dma_start(ksT[:].rearrange("p c k -> p (c k)"), ksT_d)
            wqk_sb = constp.tile([128, 2, DCH, H], BF16)
            nc.scalar.dma_start(
                wqk_sb[:].rearrange("p w c h -> p (w c h)"), wqk_d
            )
            qsT = projp.tile([128, DCH, QPC], BF16)
            nc.scalar.dma_start(qsT[:].rearrange("p c q -> p (c q)"), qsT_d)
            wq_sb = wqk_sb[:, 0]
            wk_sb = wqk_sb[:, 1]
            wv_sb = constp.tile([128, 1], FP32)
            nc.gpsimd.dma_start(wv_sb[:], wv_d)

            # masks: msk[:, j, :] = wv (x) e_j -- stationary [128h, 32m] whose
            # only nonzero column is j.
            mskd = constp.tile([128, QG, QG], FP32)
            nc.gpsimd.memset(mskd[:], 0.0)
            nc.gpsimd.affine_select(
                out=mskd[:], in_=mskd[:],
                compare_op=mybir.AluOpType.not_equal, fill=1.0,
                base=0, pattern=[[1, QG], [-1, QG]], channel_multiplier=0,
            )
            msk = constp.tile([128, QG, QG], BF16)
            nc.vector.tensor_scalar_mul(msk[:], mskd[:], wv_sb[:, 0:1])

            # PE warm-up: the tensor engine p-state ramps only after ~3us of
            # continuous work; dummy matmuls on a scratch bank while the input
            # DMAs stream get the projections to full clock.
            dum = constp.tile([128, LK], BF16)
            nc.vector.memset(dum[:], 0.0)
            ps_w = ps_warm.tile([128, LK], FP32)
            for _ in range(9):
                nc.tensor.matmul(dum_out := ps_w[:], dum[:, 0:128], dum[:],
                                 start=True, stop=True)

            # ---------------- project: qh^T = Wq^T qs^T, kh^T = Wk^T ks^T ----
            # bf16 inputs (converted host-side) run the PE at full rate.
            # kh first (every bias-add reads all of kh); the qh psum->sbuf
            # copy is split so the first groups' columns are ready early.
            ps_kh = ps_proj.tile([128, LK], FP32)
            kh_bf = projp.tile([128, LK], BF16)
            for j in range(DCH):
                nc.tensor.matmul(
                    ps_kh[:], wk_sb[:, j, :], ksT[:, j, :],
                    start=(j == 0), stop=(j == DCH - 1),
                )
            nc.vector.tensor_copy(kh_bf[:], ps_kh[:])

            ps_qh = ps_proj.tile([128, QPC], FP32)
            for j in range(DCH):
                nc.tensor.matmul(
                    ps_qh[:], wq_sb[:, j, :], qsT[:, j, :],
                    start=(j == 0), stop=(j == DCH - 1),
                )
            qh_f = projp.tile([128, QPC], FP32)
            nc.vector.tensor_copy(qh_f[:, 0:16], ps_qh[:, 0:16])
            nc.vector.tensor_copy(qh_f[:, 16:], ps_qh[:, 16:])

            # ---------------- main loop: bias-add, tanh, reduce ----------------
            psq = None
            q0 = 0
            for gsize in GROUPS:
                feats = featsp.tile([128, gsize, LK], BF16, name="feats")
                for j in range(gsize):
                    q = q0 + j
                    nc.vector.tensor_scalar_add(
                        feats[:, j, :], kh_bf[:], qh_f[:, q:q + 1]
                    )
                th = tanhp.tile([128, gsize, LK], BF16, name="th")
                nc.scalar.activation(
                    th[:], feats[:], mybir.ActivationFunctionType.Tanh
                )
                for j in range(gsize):
                    q = q0 + j
                    s0, s1 = _SEG_OF[q]
                    m = s1 - s0
                    lane = q - s0
                    if lane == 0:
                        psq = ps_out.tile([m, LK], FP32, name="psq")
                    nc.tensor.matmul(
                        psq[:], msk[:, lane, 0:m], th[:, j, :],
                        start=(lane == 0), stop=(lane == m - 1),
                    )
                    if lane == m - 1:
                        ob = outp.tile([m, LK], FP32, name="ob")
                        nc.vector.tensor_copy(ob[:], psq[:])
                        nc.sync.dma_start(out_d[s0:s1, :], ob[:])
                q0 += gsize

    nc.compile()
    return nc


_NC_CACHE = None


def _get_nc():
    global _NC_CACHE
    if _NC_CACHE is None:
        _NC_CACHE = build_nc()
    return _NC_CACHE


BF16NP = ml_dtypes.bfloat16


def _dev_layout(mT, rows):
    # [D, rows] -> [128, DCH*rows] with partition = d % 128, chunk = d // 128
    return np.ascontiguousarray(
        mT.reshape(DCH, 128, rows).transpose(1, 0, 2).reshape(128, DCH * rows)
    )


def make_in_maps(qs, ks, Wq, Wk, wv):
    qs = np.asarray(qs, dtype=np.float32)
    ks = np.asarray(ks, dtype=np.float32)
    Wqk = np.stack([np.asarray(Wq, dtype=np.float32),
                    np.asarray(Wk, dtype=np.float32)], axis=0).astype(BF16NP)
    # [2, D, H] -> [128, 2*DCH*H]
    Wqk = np.ascontiguousarray(
        Wqk.reshape(2, DCH, 128, H).transpose(2, 0, 1, 3).reshape(128, -1)
    )
    wv = np.ascontiguousarray(np.asarray(wv, dtype=np.float32))
    ksT_b = [_dev_layout(ks[b].T.astype(BF16NP), LK) for b in range(B)]
    in_maps = []
    for c in range(NCORES):
        b, half = divmod(c, 2)
        in_maps.append({
            "qsT": _dev_layout(
                qs[b, half * QPC:(half + 1) * QPC, :].T.astype(BF16NP), QPC
            ),
            "ksT": ksT_b[b],
            "Wqk": Wqk,
            "wv": wv,
        })
    return in_maps


def assemble(results):
    out = np.empty((B, LQ, LK), dtype=np.float32)
    for c in range(NCORES):
        b, half = divmod(c, 2)
        out[b, half * QPC:(half + 1) * QPC, :] = results[c]["out"]
    return out


def run(qs, ks, Wq, Wk, wv, trace=False, tmpdir=None):
    nc = _get_nc()
    in_maps = make_in_maps(qs, ks, Wq, Wk, wv)
    res = run_bass_kernel_spmd(
        nc, in_maps, core_ids=list(range(NCORES)), trace=trace, tmpdir=tmpdir
    )
    return assemble(res.results), res


def kernel(qs, ks, Wq, Wk, wv):
    out, _ = run(qs, ks, Wq, Wk, wv)
    return out


# revision 43
# speedup vs baseline: 1.0091x; 1.0048x over previous
"""Trainium2 Bass kernel for nn_AdditiveScorer (Bahdanau additive attention scores).

reference:
    q = qs @ Wq                      # [B, LQ, H]
    k = ks @ Wk                      # [B, LK, H]
    scores[b,q,k] = sum_h wv[h] * tanh(q[b,q,h] + k[b,k,h])   # [B, LQ, LK]

Shapes (hardcoded): B=4, LQ=LK=D=512, H=128.

Sharding: B*LQ = 2048 query rows split across 8 cores -> 256 rows/core.
Core c handles batch b = c//2, query rows [256*(c%2), 256*(c%2+1)).
Each core only needs its batch's ks (replicated host-side to the core pair).
No collectives: embarrassingly parallel; host gathers per-core score slices.
qs/ks are transposed host-side (pure layout prep) so no PE transposes are
needed on device.

Per-core pipeline (engines run concurrently, ACT tanh is the roofline:
~113us busy of ~137us total):
  - PE: project to qh^T [H=128p, 256], kh^T [H=128p, 512] (bf16, f32 accum)
  - DVE: per query q, feats[:, j, :] = kh^T + qh^T[:, q]
    (tensor_scalar add with per-partition scalar operand, bf16 2x mode)
  - ACT: tanh over [128, G*512] tiles (1 elem/lane/cycle, groups ramp
    4,4,8,16...16,8,4,4 to hide prologue latency and shorten the tail)
  - PE: scores row q = wv^T @ tanh_feats via masked stationary wv (x) e_lane;
    a segment of queries accumulates into one PSUM [seg, 512] tile (rows
    other than `lane` accumulate exact zeros)
  - DVE: PSUM->SBUF copy per segment, DMA out
"""

import ml_dtypes
import numpy as np

import concourse.tile as tile
from concourse import bacc, mybir
from concourse.bass_utils import run_bass_kernel_spmd

FP32 = mybir.dt.float32
BF16 = mybir.dt.bfloat16

B, LQ, LK, D, H = 4, 512, 512, 512, 128
NCORES = 8
QPC = B * LQ // NCORES      # 256 query rows per core
DCH = D // 128              # contraction chunks (4)
QG = 32                     # queries per PSUM accumulation group

# ACT group sizes: small leading groups start the tanh pipeline early (less
# prologue latency), wide middles amortize the per-instruction overhead, small
# tail groups shorten the post-last-tanh PE drain.
GROUPS = [4, 4, 8] + [16] * 14 + [8, 4, 4]
assert sum(GROUPS) == QPC

# PSUM accumulation segments (query ranges). The last two are 16-wide so the
# final output flush starts earlier.
SEGS = [(s, s + 32) for s in range(0, 224, 32)] + [(224, 240), (240, 256)]
_SEG_OF = {}
for _s0, _s1 in SEGS:
    for _q in range(_s0, _s1):
        _SEG_OF[_q] = (_s0, _s1)


def build_nc():
    nc = bacc.Bacc("TRN2", target_bir_lowering=False, debug=False, num_devices=NCORES)

    # inputs arrive already in device layout: [partition, flat free dim]
    qsT_d = nc.dram_tensor("qsT", [128, DCH * QPC], BF16, kind="ExternalInput").ap()
    ksT_d = nc.dram_tensor("ksT", [128, DCH * LK], BF16, kind="ExternalInput").ap()
    wqk_d = nc.dram_tensor("Wqk", [128, 2 * DCH * H], BF16, kind="ExternalInput").ap()
    wv_d = nc.dram_tensor("wv", [H, 1], FP32, kind="ExternalInput").ap()
    out_d = nc.dram_tensor("out", [QPC, LK], FP32, kind="ExternalOutput").ap()

    with tile.TileContext(nc) as tc:
        with (
            tc.tile_pool(name="const", bufs=1) as constp,
            tc.tile_pool(name="proj", bufs=1) as projp,
            tc.tile_pool(name="feats", bufs=4) as featsp,
            tc.tile_pool(name="tanhp", bufs=3) as tanhp,
            tc.tile_pool(name="outs", bufs=2) as outp,
            tc.tile_pool(name="ps_proj", bufs=1, space="PSUM") as ps_proj,
            tc.tile_pool(name="ps_out", bufs=2, space="PSUM") as ps_out,
            tc.tile_pool(name="ps_warm", bufs=1, space="PSUM") as ps_warm,
        ):
            # ---------------- loads (triggers spread across engines) --------
            # ksT is the critical path (kh feeds every bias-add); issue first.
            ksT = projp.tile([128, DCH, LK], BF16)
            nc.sync.dma_start(ksT[:].rearrange("p c k -> p (c k)"), ksT_d)
            qsT = projp.tile([128, DCH, QPC], BF16)
            nc.scalar.dma_start(qsT[:].rearrange("p c q -> p (c q)"), qsT_d)
            wqk_sb = constp.tile([128, 2, DCH, H], BF16)
            nc.gpsimd.dma_start(
                wqk_sb[:].rearrange("p w c h -> p (w c h)"), wqk_d
            )
            wq_sb = wqk_sb[:, 0]
            wk_sb = wqk_sb[:, 1]
            wv_sb = constp.tile([128, 1], FP32)
            nc.gpsimd.dma_start(wv_sb[:], wv_d)

            # masks: msk[:, j, :] = wv (x) e_j -- stationary [128h, 32m] whose
            # only nonzero column is j.
            mskd = constp.tile([128, QG, QG], FP32)
            nc.gpsimd.memset(mskd[:], 0.0)
            nc.gpsimd.affine_select(
                out=mskd[:], in_=mskd[:],
                compare_op=mybir.AluOpType.not_equal, fill=1.0,
                base=0, pattern=[[1, QG], [-1, QG]], channel_multiplier=0,
            )
            msk = constp.tile([128, QG, QG], BF16)
            nc.vector.tensor_scalar_mul(msk[:], mskd[:], wv_sb[:, 0:1])

            # PE warm-up: the tensor engine p-state ramps only after ~3us of
            # continuous work; dummy matmuls on a scratch bank while the input
            # DMAs stream get the projections to full clock.
            dum = constp.tile([128, LK], BF16)
            nc.vector.memset(dum[:], 0.0)
            ps_w = ps_warm.tile([128, LK], FP32)
            for _ in range(9):
                nc.tensor.matmul(dum_out := ps_w[:], dum[:, 0:128], dum[:],
                                 start=True, stop=True)

            # ---------------- project: qh^T = Wq^T qs^T, kh^T = Wk^T ks^T ----
            # bf16 inputs (converted host-side) run the PE at full rate.
            # kh first (every bias-add reads all of kh); the qh psum->sbuf
            # copy is split so the first groups' columns are ready early.
            ps_kh = ps_proj.tile([128, LK], FP32)
            kh_bf = projp.tile([128, LK], BF16)
            for j in range(DCH):
                nc.tensor.matmul(
                    ps_kh[:], wk_sb[:, j, :], ksT[:, j, :],
                    start=(j == 0), stop=(j == DCH - 1),
                )
            nc.vector.tensor_copy(kh_bf[:], ps_kh[:])

            ps_qh = ps_proj.tile([128, QPC], FP32)
            for j in range(DCH):
                nc.tensor.matmul(
                    ps_qh[:], wq_sb[:, j, :], qsT[:, j, :],
                    start=(j == 0), stop=(j == DCH - 1),
                )
            qh_f = projp.tile([128, QPC], FP32)
            nc.vector.tensor_copy(qh_f[:, 0:16], ps_qh[:, 0:16])
            nc.vector.tensor_copy(qh_f[:, 16:], ps_qh[:, 16:])

            # ---------------- main loop: bias-add, tanh, reduce ----------------
            psq = None
            q0 = 0
            for gsize in GROUPS:
                feats = featsp.tile([128, gsize, LK], BF16, name="feats")
                for j in range(gsize):
                    q = q0 + j
                    nc.vector.tensor_scalar_add(
                        feats[:, j, :], kh_bf[:], qh_f[:, q:q + 1]
                    )
                th = tanhp.tile([128, gsize, LK], BF16, name="th")
                nc.scalar.activation(
                    th[:], feats[:], mybir.ActivationFunctionType.Tanh
                )
                for j in range(gsize):
                    q = q0 + j
                    s0, s1 = _SEG_OF[q]
                    m = s1 - s0
                    lane = q - s0
                    if lane == 0:
                        psq = ps_out.tile([m, LK], FP32, name="psq")
                    nc.tensor.matmul(
                        psq[:], msk[:, lane, 0:m], th[:, j, :],
                        start=(lane == 0), stop=(lane == m - 1),
                    )
                    if lane == m - 1:
                        ob = outp.tile([m, LK], FP32, name="ob")
                        nc.vector.tensor_copy(ob[:], psq[:])
                        nc.sync.dma_start(out_d[s0:s1, :], ob[:])
                q0 += gsize

    nc.compile()
    return nc


_NC_CACHE = None


def _get_nc():
    global _NC_CACHE
    if _NC_CACHE is None:
        _NC_CACHE = build_nc()
    return _NC_CACHE


BF16NP = ml_dtypes.bfloat16


def _dev_layout(mT, rows):
    # [D, rows] -> [128, DCH*rows] with partition = d % 128, chunk = d // 128
    return np.ascontiguousarray(
        mT.reshape(DCH, 128, rows).transpose(1, 0, 2).reshape(128, DCH * rows)
    )


def make_in_maps(qs, ks, Wq, Wk, wv):
    qs = np.asarray(qs, dtype=np.float32)
    ks = np.asarray(ks, dtype=np.float32)
    Wqk = np.stack([np.asarray(Wq, dtype=np.float32),
                    np.asarray(Wk, dtype=np.float32)], axis=0).astype(BF16NP)
    # [2, D, H] -> [128, 2*DCH*H]
    Wqk = np.ascontiguousarray(
        Wqk.reshape(2, DCH, 128, H).transpose(2, 0, 1, 3).reshape(128, -1)
    )
    wv = np.ascontiguousarray(np.asarray(wv, dtype=np.float32))
    ksT_b = [_dev_layout(ks[b].T.astype(BF16NP), LK) for b in range(B)]
    in_maps = []
    for c in range(NCORES):
        b, half = divmod(c, 2)
        in_maps.append({
            "qsT": _dev_layout(
                qs[b, half * QPC:(half + 1) * QPC, :].T.astype(BF16NP), QPC
            ),
            "ksT": ksT_b[b],
            "Wqk": Wqk,
            "wv": wv,
        })
    return in_maps


def assemble(results):
    out = np.empty((B, LQ, LK), dtype=np.float32)
    for c in range(NCORES):
        b, half = divmod(c, 2)
        out[b, half * QPC:(half + 1) * QPC, :] = results[c]["out"]
    return out


def run(qs, ks, Wq, Wk, wv, trace=False, tmpdir=None):
    nc = _get_nc()
    in_maps = make_in_maps(qs, ks, Wq, Wk, wv)
    res = run_bass_kernel_spmd(
        nc, in_maps, core_ids=list(range(NCORES)), trace=trace, tmpdir=tmpdir
    )
    return assemble(res.results), res


def kernel(qs, ks, Wq, Wk, wv):
    out, _ = run(qs, ks, Wq, Wk, wv)
    return out
